# revision 22
# baseline (speedup 1.0000x reference)
"""Sliding-window causal self-attention (GQA + QK L2-norm + RoPE) on 8 TRN2 cores.

Sharding: data-parallel over the sequence dim. Core c computes output rows
[c*256, (c+1)*256). It loads x rows [c*256-512, (c+1)*256) (sliding-window
halo, zero-padded below row 0) and recomputes K/V projections for the halo
locally, so no collectives are needed.

Per-core layouts (all "T-layouts", contraction dim on SBUF partitions):
  xT   [C, 768]     x-halo transposed (host-side)
  q    qT [o=128(2 heads x 64), ob=8, t=256]     (projection emits transposed)
  k    kT [o=128(2 kv heads x 64), kob=2, t=768]
  v    natural [t=128, tb=6, 4*65] (65-strided with a ones column for rowsums)
  scores  sT [j, i] per (head, key-block)  -> softmax denominator comes out of
          the PV matmul via the ones column; normalization is applied to yT.
  y    yT [f=128, fb=8, t=256] feeds c_proj directly.

Softmax skips the max-subtraction: q,k are L2-normalized so |score| <= 0.125
and exp never overflows. Masked positions get -30000 before exp -> exactly 0.
"""

import os

import numpy as np

import concourse.bacc as bacc
import concourse.mybir as mybir
import concourse.tile as tile
from concourse.bass_utils import run_bass_kernel_spmd

F32 = mybir.dt.float32
F32R = mybir.dt.float32r
AF = mybir.ActivationFunctionType
OP = mybir.AluOpType

T, C = 2048, 1024
H, KVH, D = 16, 4, 64
WIN = 512
NCORES = 8
R = T // NCORES          # 256 query rows per core
NKV = R + WIN            # 768 local kv rows
NKB = NKV // 128         # 6 key blocks per strip
EPS = 1e-6
NEG = -240000.0          # additive mask value, pre-scaled by 8 (exp scale=1/8)

# Q-head placement: block b holds heads (HPERM[b][0], HPERM[b][1]) on partition
# halves 0/1. Chosen so a head's partition half equals its kv head's partition
# half ((h//4) % 2), letting the score matmul read q and k at the same base
# partition. Applied host-side as a column permutation of Wq.T / row
# permutation of Wproj.T.
HPERM = [[0, 4], [1, 5], [2, 6], [3, 7], [8, 12], [9, 13], [10, 14], [11, 15]]


def _rope_tables():
    # mirrors reference._rope_tables in float32
    theta = 1.0 / (10000.0 ** (np.arange(0, D, 2, dtype=np.float32) / D))
    pos = np.arange(T, dtype=np.float32)
    freqs = np.outer(pos, theta)                                   # [T, 32]
    cos = np.concatenate([np.cos(freqs), np.cos(freqs)], axis=-1)  # [T, 64]
    sin = np.concatenate([np.sin(freqs), np.sin(freqs)], axis=-1)
    return cos.astype(np.float32), sin.astype(np.float32)


def _emit(nc):
    xT = nc.dram_tensor("xT", [C, NKV], F32R, kind="ExternalInput").ap()
    wqT = nc.dram_tensor("wqT", [C, C], F32R, kind="ExternalInput").ap()
    wkvT = nc.dram_tensor("wkvT", [C, 512], F32R, kind="ExternalInput").ap()
    wpT = nc.dram_tensor("wpT", [C, C], F32R, kind="ExternalInput").ap()
    cosq = nc.dram_tensor("cosq", [128, R], F32, kind="ExternalInput").ap()
    sinq = nc.dram_tensor("sinq", [128, R], F32, kind="ExternalInput").ap()
    cosk = nc.dram_tensor("cosk", [128, NKV], F32, kind="ExternalInput").ap()
    sink = nc.dram_tensor("sink", [128, NKV], F32, kind="ExternalInput").ap()
    mcomb = nc.dram_tensor("mcomb", [128, 2, 512], F32, kind="ExternalInput").ap()
    jb = nc.dram_tensor("jb", [128, NKB], F32, kind="ExternalInput").ap()
    eee = nc.dram_tensor("eee", [128, 2], F32R, kind="ExternalInput").ap()
    e2 = nc.dram_tensor("e2", [2, 128], F32R, kind="ExternalInput").ap()
    ones64 = nc.dram_tensor("ones64", [1, 64], F32R, kind="ExternalInput").ap()
    rotm = nc.dram_tensor("rotm", [128, 128], F32R, kind="ExternalInput").ap()
    out = nc.dram_tensor("out", [R, C], F32, kind="ExternalOutput").ap()
    dbg = {}
    if os.environ.get("KDEBUG"):
        for nm, shp in [("dq2", [128, 8, R]), ("dk2", [128, 2, NKV]),
                        ("dvsb", [128, NKB, KVH * 65]), ("dyT", [128, 8, R]),
                        ("du", [16, 128, 2 * R])]:
            dbg[nm] = nc.dram_tensor(nm, shp, F32, kind="ExternalOutput").ap()

    with tile.TileContext(nc) as tc:
        with (
            tc.tile_pool(name="wp", bufs=1) as wp,
            tc.tile_pool(name="acts", bufs=1) as acts,
            tc.tile_pool(name="sp", bufs=2) as sp,
        ):
            # ---- persistent loads ----
            xT_sb = wp.tile([128, 8, NKV], F32R, name="xT_sb")
            nc.sync.dma_start(out=xT_sb[:], in_=xT.rearrange("(a p) t -> p a t", p=128))
            wqT_sb = wp.tile([128, 8, C], F32R, name="wqT_sb")
            nc.sync.dma_start(out=wqT_sb[:], in_=wqT.rearrange("(a p) o -> p a o", p=128))
            wkvT_sb = wp.tile([128, 8, 512], F32R, name="wkvT_sb")
            nc.sync.dma_start(out=wkvT_sb[:], in_=wkvT.rearrange("(a p) o -> p a o", p=128))
            wpT_sb = wp.tile([128, 8, C], F32R, name="wpT_sb")
            nc.sync.dma_start(out=wpT_sb[:], in_=wpT.rearrange("(a p) o -> p a o", p=128))
            cosq_sb = wp.tile([128, R], F32, name="cosq_sb")
            nc.sync.dma_start(out=cosq_sb[:], in_=cosq)
            sinq_sb = wp.tile([128, R], F32, name="sinq_sb")
            nc.sync.dma_start(out=sinq_sb[:], in_=sinq)
            cosk_sb = wp.tile([128, NKV], F32, name="cosk_sb")
            nc.sync.dma_start(out=cosk_sb[:], in_=cosk)
            sink_sb = wp.tile([128, NKV], F32, name="sink_sb")
            nc.sync.dma_start(out=sink_sb[:], in_=sink)
            mcomb_sb = wp.tile([128, 2, 512], F32, name="mcomb_sb")
            nc.sync.dma_start(out=mcomb_sb[:], in_=mcomb)
            jb_sb = wp.tile([128, NKB], F32, name="jb_sb")
            nc.sync.dma_start(out=jb_sb[:], in_=jb)
            eee_sb = wp.tile([128, 2], F32R, name="eee_sb")
            nc.sync.dma_start(out=eee_sb[:], in_=eee)
            e2_sb = wp.tile([2, 128], F32R, name="e2_sb")
            nc.sync.dma_start(out=e2_sb[:], in_=e2)
            ones64_sb = wp.tile([1, 64], F32R, name="ones64_sb")
            nc.sync.dma_start(out=ones64_sb[:], in_=ones64)
            rotm_sb = wp.tile([128, 128], F32R, name="rotm_sb")
            nc.sync.dma_start(out=rotm_sb[:], in_=rotm)

            # ---- persistent activations ----
            q2 = acts.tile([128, 8, R], F32R, name="q2")
            k2 = acts.tile([128, 2, NKV], F32R, name="k2")
            vsb = acts.tile([128, NKB, KVH * 65], F32R, name="vsb")
            yT = acts.tile([128, 8, R], F32R, name="yT")
            outsb = acts.tile([128, 2, C], F32, name="outsb")

            vsb4 = vsb.rearrange("p b (h e) -> p b h e", e=65)
            ONE_F32 = 1065353216  # 1.0f bit pattern; f32r memset is not a valid ISA op
            for _tb in range(NKB):
                for _kvh in range(KVH):
                    nc.gpsimd.memset(
                        vsb4[:, _tb, _kvh, 64:65].bitcast(mybir.dt.uint32), ONE_F32)

            # =========== phase A: projections + qk-norm + rope ===========
            with tc.tile_pool(name="ppA", bufs=2, space="PSUM") as ppA:

                def qk_post(psum_x, n, cos_sb, sin_sb, dst):
                    """psum_x [128, n] raw (q or k)T projection; writes
                    rope(norm(.)) into dst [128, n]."""
                    t_sb = sp.tile([128, n], F32R, tag="tqk", name="t_sb")
                    nc.scalar.copy(t_sb[:], psum_x[:])
                    sq = sp.tile([128, n], F32R, tag="tsq", name="sq")
                    nc.scalar.activation(sq[:], t_sb[:], AF.Square)
                    ss = ppA.tile([2, n], F32, tag="ss", name="ss")
                    for j0 in range(0, n, 512):
                        j1 = min(j0 + 512, n)
                        nc.tensor.matmul(
                            ss[:, j0:j1],
                            eee_sb[:],
                            sq[:, j0:j1],
                            start=True, stop=True)
                    nrm = sp.tile([2, n], F32, tag="nrm", bufs=1, name="nrm")
                    nc.scalar.activation(nrm[:], ss[:], AF.Sqrt)
                    nc.vector.tensor_scalar_add(nrm[:], nrm[:], EPS)
                    rnf = sp.tile([2, n], F32, tag="trn", bufs=1, name="rnf")
                    nc.vector.reciprocal(rnf[:], nrm[:])
                    rn = sp.tile([2, n], F32R, tag="trnr", bufs=1, name="rn")
                    nc.vector.tensor_copy(rn[:], rnf[:])
                    # broadcast the two per-head norm rows across partition
                    # halves via a k=2 matmul (gpsimd partition_broadcast is
                    # unreliable on HW)
                    RN = ppA.tile([128, n], F32, tag="pj", name="RN")
                    for j0 in range(0, n, 512):
                        j1 = min(j0 + 512, n)
                        nc.tensor.matmul(
                            RN[:, j0:j1], e2_sb[:], rn[:, j0:j1],
                            start=True, stop=True)
                    rot = ppA.tile([128, n], F32, tag="pj", name="rot")
                    for j0 in range(0, n, 512):
                        j1 = min(j0 + 512, n)
                        nc.tensor.matmul(
                            rot[:, j0:j1],
                            rotm_sb[:],
                            t_sb[:, j0:j1],
                            start=True, stop=True)
                    t1 = sp.tile([128, n], F32, tag="tr1", name="t1")
                    nc.vector.tensor_mul(t1[:], rot[:], sin_sb[:])
                    t2 = sp.tile([128, n], F32, tag="tr2", name="t2")
                    nc.vector.tensor_mul(t2[:], t_sb[:], cos_sb[:])
                    nc.vector.tensor_add(t1[:], t1[:], t2[:])
                    nc.vector.tensor_mul(dst, t1[:], RN[:])

                # Q projection -> q2 (transposed layout, per o-block)
                for ob in range(8):
                    pq = ppA.tile([128, R], F32, tag="pj", name="pq")
                    for ci in range(8):
                        nc.tensor.matmul(
                            pq[:],
                            wqT_sb[:, ci, ob * 128:(ob + 1) * 128],
                            xT_sb[:, ci, WIN:NKV],
                            start=(ci == 0), stop=(ci == 7))
                    qk_post(pq, R, cosq_sb, sinq_sb, q2[:, ob, :])

                # K projection -> k2
                for kob in range(2):
                    pk = ppA.tile([128, NKV], F32, tag="pj", name="pk")
                    for ci in range(8):
                        for j0 in range(0, NKV, 512):
                            j1 = min(j0 + 512, NKV)
                            nc.tensor.matmul(
                                pk[:, j0:j1],
                                wkvT_sb[:, ci, 256 + kob * 128:256 + (kob + 1) * 128],
                                xT_sb[:, ci, j0:j1],
                                start=(ci == 0), stop=(ci == 7))
                    qk_post(pk, NKV, cosk_sb, sink_sb, k2[:, kob, :])

                # V projection -> vsb (natural layout)
                for tb in range(NKB):
                    pvp = ppA.tile([128, 256], F32, tag="pj", name="pvp")
                    for ci in range(8):
                        nc.tensor.matmul(
                            pvp[:],
                            xT_sb[:, ci, tb * 128:(tb + 1) * 128],
                            wkvT_sb[:, ci, 0:256],
                            start=(ci == 0), stop=(ci == 7))
                    nc.vector.tensor_copy(
                        vsb4[:, tb, :, 0:64],
                        pvp.rearrange("p (h e) -> p h e", e=64))

            # =========== phase B: attention + c_proj ===========
            with tc.tile_pool(name="ppB", bufs=1, space="PSUM") as ppB:
                for b in range(8):
                  for s in range(2):
                    h = HPERM[b][s]
                    kvh = h // 4
                    kob, poff = kvh // 2, s * 64
                    qsl = q2[s * 64:s * 64 + 64, b, :]
                    py = ppB.tile([65, R], F32, tag="pv", bufs=2, name="py")
                    for pr in range(3):          # kb pairs (0,1) (2,3) (4,5)
                        ps = ppB.tile([128, 2, R], F32, tag="ps", bufs=3, name="ps")
                        for kl in range(2):
                            kb = pr * 2 + kl
                            nc.tensor.matmul(
                                ps[:, kl, :],
                                k2[poff:poff + 64, kob, kb * 128:(kb + 1) * 128],
                                qsl,
                                start=True, stop=True)
                        psf = ps.rearrange("p a i -> p (a i)")
                        u = sp.tile([128, 2 * R], F32R, tag="uT", bufs=2, name="u")
                        if pr == 1:
                            # inside the window: only the per-key padding bias
                            # (nonzero on cores 0-1 only) applies
                            for kl in range(2):
                                kb = pr * 2 + kl
                                nc.scalar.activation(
                                    u[:, kl * R:(kl + 1) * R], ps[:, kl, :],
                                    AF.Exp, bias=jb_sb[:, kb:kb + 1], scale=0.125)
                        else:
                            stg = sp.tile([128, 2 * R], F32, tag="stg", name="stg")
                            nc.vector.tensor_add(stg[:], psf, mcomb_sb[:, pr // 2, :])
                            nc.scalar.activation(u[:], stg[:], AF.Exp, scale=0.125)
                        if dbg and pr == 0:
                            nc.sync.dma_start(out=dbg["du"][h].bitcast(F32R), in_=u[:])
                        for kl in range(2):
                            kb = pr * 2 + kl
                            nc.tensor.matmul(
                                py[:],
                                vsb4[:, kb, kvh, :],
                                u[:, kl * R:(kl + 1) * R],
                                start=(kb == 0), stop=(kb == NKB - 1))
                    rsrf = sp.tile([1, R], F32, tag="rsrf", name="rsrf")
                    nc.vector.reciprocal(rsrf[:], py[64:65, :])
                    rsr = sp.tile([1, R], F32R, tag="rsr", name="rsr")
                    nc.vector.tensor_copy(rsr[:], rsrf[:])
                    RRp = ppB.tile([64, R], F32, tag="rr", bufs=1, name="RRp")
                    nc.tensor.matmul(RRp[:], ones64_sb[:], rsr[:],
                                     start=True, stop=True)
                    RRb = sp.tile([64, R], F32, tag="RRb", name="RRb")
                    nc.scalar.copy(RRb[:], RRp[:])
                    nc.vector.tensor_mul(
                        yT[s * 64:s * 64 + 64, b, :],
                        py[0:64, :], RRb[:])

                if dbg:
                    nc.sync.dma_start(out=dbg["dq2"].bitcast(F32R), in_=q2[:])
                    nc.sync.dma_start(out=dbg["dk2"].bitcast(F32R), in_=k2[:])
                    nc.sync.dma_start(out=dbg["dvsb"].bitcast(F32R), in_=vsb[:])
                    nc.sync.dma_start(out=dbg["dyT"].bitcast(F32R), in_=yT[:])
                # c_proj
                for itb in range(2):
                    for oh in range(2):
                        po = ppB.tile([128, 512], F32, tag="co", bufs=2, name="po")
                        for fb in range(8):
                            nc.tensor.matmul(
                                po[:],
                                yT[:, fb, itb * 128:(itb + 1) * 128],
                                wpT_sb[:, fb, oh * 512:(oh + 1) * 512],
                                start=(fb == 0), stop=(fb == 7))
                        nc.scalar.copy(outsb[:, itb, oh * 512:(oh + 1) * 512], po[:])
                    nc.sync.dma_start(
                        out=out[itb * 128:(itb + 1) * 128, :],
                        in_=outsb[:, itb, :])
    return nc


_CACHE = {}


def _build():
    if "nc" not in _CACHE:
        nc = bacc.Bacc(trn_type="TRN2", target_bir_lowering=False, debug=False)
        _emit(nc)
        nc.compile()
        _CACHE["nc"] = nc
    return _CACHE["nc"]


def _make_core_inputs(x, Wq, Wkv, Wproj):
    """Host-side prep: per-core shards + constant tables (all float32)."""
    x2 = np.ascontiguousarray(np.asarray(x, dtype=np.float32).reshape(T, C))
    # head permutation (see HPERM): feature index new = (2b+s)*64+d <- old h*64+d
    perm = np.concatenate([
        np.arange(64) + HPERM[b][s] * 64
        for b in range(8) for s in range(2)])
    wqT = np.ascontiguousarray(np.asarray(Wq, dtype=np.float32).T[:, perm])
    # Wkv rows: [k(4h x 64) | v(4h x 64)]; wkvT columns reordered to [v | k]
    wkv = np.asarray(Wkv, dtype=np.float32)
    wkvT = np.ascontiguousarray(np.concatenate([wkv[256:512], wkv[0:256]], axis=0).T)
    wpT = np.ascontiguousarray(np.asarray(Wproj, dtype=np.float32).T[perm, :])
    cos, sin = _rope_tables()

    # rotate-half matrix (lhsT layout [k, m]): out[m] = sum_k rotm[k, m] * q[k]
    rot = np.zeros((64, 64), dtype=np.float32)
    for m in range(32):
        rot[m + 32, m] = -1.0       # out[m] = -q[m+32]
        rot[m, m + 32] = 1.0        # out[m+32] = q[m]
    rotm = np.zeros((128, 128), dtype=np.float32)
    rotm[0:64, 0:64] = rot
    rotm[64:128, 64:128] = rot

    eee = np.zeros((128, 2), dtype=np.float32)
    eee[0:64, 0] = 1.0
    eee[64:128, 1] = 1.0
    e2 = np.zeros((2, 128), dtype=np.float32)
    e2[0, 0:64] = 1.0
    e2[1, 64:128] = 1.0
    ones64 = np.ones((1, 64), dtype=np.float32)

    jj = np.arange(128)[:, None]
    ii = np.arange(128)[None, :]
    triA = np.where(jj > ii, 0.0, NEG).astype(np.float32)   # valid: j' > i'
    triB = np.where(jj <= ii, 0.0, NEG).astype(np.float32)  # valid: j' <= i'
    full = np.zeros((128, 128), dtype=np.float32)
    negf = np.full((128, 128), NEG, dtype=np.float32)

    in_maps = []
    for c in range(NCORES):
        qs = c * R
        ks = qs - WIN
        pad = max(0, -ks)            # leading zero-padded kv rows
        xpad = np.zeros((NKV, C), dtype=np.float32)
        xpad[pad:, :] = x2[ks + pad:qs + R, :]
        xTc = np.ascontiguousarray(xpad.T)

        cq = np.ascontiguousarray(np.tile(cos[qs:qs + R].T, (2, 1)))   # [128, R]
        sq_ = np.ascontiguousarray(np.tile(sin[qs:qs + R].T, (2, 1)))
        ck = np.zeros((128, NKV), dtype=np.float32)
        sk = np.zeros((128, NKV), dtype=np.float32)
        ck[:, pad:] = np.tile(cos[ks + pad:qs + R].T, (2, 1))
        sk[:, pad:] = np.tile(sin[ks + pad:qs + R].T, (2, 1))

        # combined additive masks for kb pairs (0,1) and (4,5), jbias folded in
        jb8 = np.zeros((128, NKB), dtype=np.float32)
        for kb in range(NKB):
            jb8[:, kb] = np.where(kb * 128 + np.arange(128) < pad, NEG, 0.0)
        m = np.zeros((128, 2, 2, 2, 128), dtype=np.float32)  # [j, pr, kl, qb, i]
        m[:, 0, 0, 0] = triA + jb8[:, 0:1]
        m[:, 0, 0, 1] = negf
        m[:, 0, 1, 0] = full + jb8[:, 1:2]
        m[:, 0, 1, 1] = triA + jb8[:, 1:2]
        m[:, 1, 0, 0] = triB
        m[:, 1, 0, 1] = full
        m[:, 1, 1, 0] = negf
        m[:, 1, 1, 1] = triB
        mcomb = np.ascontiguousarray(m.reshape(128, 2, 512))
        jbu = (jb8 / 8.0).astype(np.float32)   # unscaled bias for the ACT path

        in_maps.append({
            "xT": xTc, "wqT": wqT, "wkvT": wkvT, "wpT": wpT,
            "cosq": cq, "sinq": sq_, "cosk": ck, "sink": sk,
            "mcomb": mcomb, "jb": jbu, "eee": eee, "e2": e2, "ones64": ones64, "rotm": rotm,
        })
    return in_maps


def _run(x, Wq, Wkv, Wproj, trace=False):
    nc = _build()
    in_maps = _make_core_inputs(x, Wq, Wkv, Wproj)
    res = run_bass_kernel_spmd(nc, in_maps, core_ids=list(range(NCORES)), trace=trace)
    outs = [res.results[c]["out"] for c in range(NCORES)]
    full = np.concatenate(outs, axis=0).reshape(1, T, C).astype(np.float32)
    return full, res


def kernel(x, Wq, Wkv, Wproj):
    full, _ = _run(x, Wq, Wkv, Wproj, trace=False)
    return full


# revision 25
# speedup vs baseline: 1.2317x; 1.2317x over previous
"""Sliding-window causal self-attention (GQA + QK L2-norm + RoPE) on 8 TRN2 cores.

Sharding: data-parallel over the sequence dim. Core c computes output rows
[c*256, (c+1)*256). It loads x rows [c*256-512, (c+1)*256) (sliding-window
halo, zero-padded below row 0) and recomputes K/V projections for the halo
locally, so no collectives are needed.

Per-core layouts (all "T-layouts", contraction dim on SBUF partitions):
  xT   [C, 768]     x-halo transposed (host-side)
  q    qT [o=128(2 heads x 64), ob=8, t=256]     (projection emits transposed)
  k    kT [o=128(2 kv heads x 64), kob=2, t=768]
  v    natural [t=128, tb=6, 4*65] (65-strided with a ones column for rowsums)
  scores  sT [j, i] per (head, key-block)  -> softmax denominator comes out of
          the PV matmul via the ones column; normalization is applied to yT.
  y    yT [f=128, fb=8, t=256] feeds c_proj directly.

Softmax skips the max-subtraction: q,k are L2-normalized so |score| <= 0.125
and exp never overflows. Masked positions get -30000 before exp -> exactly 0.
"""

import os

import numpy as np

import concourse.bacc as bacc
import concourse.mybir as mybir
import concourse.tile as tile
from concourse.bass_utils import run_bass_kernel_spmd

F32 = mybir.dt.float32
F32R = mybir.dt.float32r
AF = mybir.ActivationFunctionType
OP = mybir.AluOpType

T, C = 2048, 1024
H, KVH, D = 16, 4, 64
WIN = 512
NCORES = 8
R = T // NCORES          # 256 query rows per core
NKV = R + WIN            # 768 local kv rows
NKB = NKV // 128         # 6 key blocks per strip
EPS = 1e-6
NEG = -240000.0          # additive mask value, pre-scaled by 8 (exp scale=1/8)

# Q-head placement: block b holds heads (HPERM[b][0], HPERM[b][1]) on partition
# halves 0/1. Chosen so a head's partition half equals its kv head's partition
# half ((h//4) % 2), letting the score matmul read q and k at the same base
# partition. Applied host-side as a column permutation of Wq.T / row
# permutation of Wproj.T.
HPERM = [[0, 4], [1, 5], [2, 6], [3, 7], [8, 12], [9, 13], [10, 14], [11, 15]]


def _rope_tables():
    # mirrors reference._rope_tables in float32
    theta = 1.0 / (10000.0 ** (np.arange(0, D, 2, dtype=np.float32) / D))
    pos = np.arange(T, dtype=np.float32)
    freqs = np.outer(pos, theta)                                   # [T, 32]
    cos = np.concatenate([np.cos(freqs), np.cos(freqs)], axis=-1)  # [T, 64]
    sin = np.concatenate([np.sin(freqs), np.sin(freqs)], axis=-1)
    return cos.astype(np.float32), sin.astype(np.float32)


def _emit(nc):
    xT = nc.dram_tensor("xT", [C, NKV], F32R, kind="ExternalInput").ap()
    wqT = nc.dram_tensor("wqT", [C, C], F32R, kind="ExternalInput").ap()
    wkvT = nc.dram_tensor("wkvT", [C, 512], F32R, kind="ExternalInput").ap()
    wpT = nc.dram_tensor("wpT", [C, C], F32R, kind="ExternalInput").ap()
    cosq = nc.dram_tensor("cosq", [128, R], F32, kind="ExternalInput").ap()
    sinq = nc.dram_tensor("sinq", [128, R], F32, kind="ExternalInput").ap()
    cosk = nc.dram_tensor("cosk", [128, NKV], F32, kind="ExternalInput").ap()
    sink = nc.dram_tensor("sink", [128, NKV], F32, kind="ExternalInput").ap()
    mcomb = nc.dram_tensor("mcomb", [128, 2, 512], F32, kind="ExternalInput").ap()
    jb = nc.dram_tensor("jb", [128, NKB], F32, kind="ExternalInput").ap()
    eee = nc.dram_tensor("eee", [128, 2], F32R, kind="ExternalInput").ap()
    e2 = nc.dram_tensor("e2", [2, 128], F32R, kind="ExternalInput").ap()
    ones64 = nc.dram_tensor("ones64", [1, 64], F32R, kind="ExternalInput").ap()
    epsb = nc.dram_tensor("epsb", [2, 1], F32, kind="ExternalInput").ap()
    rotm = nc.dram_tensor("rotm", [128, 128], F32R, kind="ExternalInput").ap()
    out = nc.dram_tensor("out", [R, C], F32, kind="ExternalOutput").ap()
    dbg = {}
    if os.environ.get("KDEBUG"):
        for nm, shp in [("dq2", [128, 8, R]), ("dk2", [128, 2, NKV]),
                        ("dvsb", [128, NKB, KVH * 65]), ("dyT", [128, 8, R]),
                        ("du", [16, 128, 2 * R])]:
            dbg[nm] = nc.dram_tensor(nm, shp, F32, kind="ExternalOutput").ap()

    with tile.TileContext(nc) as tc:
        with (
            tc.tile_pool(name="wp", bufs=1) as wp,
            tc.tile_pool(name="acts", bufs=1) as acts,
            tc.tile_pool(name="sp", bufs=2) as sp,
        ):
            # ---- persistent loads ----
            xT_sb = wp.tile([128, 8, NKV], F32R, name="xT_sb")
            nc.sync.dma_start(out=xT_sb[:], in_=xT.rearrange("(a p) t -> p a t", p=128))
            wqT_sb = wp.tile([128, 8, C], F32R, name="wqT_sb")
            nc.sync.dma_start(out=wqT_sb[:], in_=wqT.rearrange("(a p) o -> p a o", p=128))
            wkvT_sb = wp.tile([128, 8, 512], F32R, name="wkvT_sb")
            nc.sync.dma_start(out=wkvT_sb[:], in_=wkvT.rearrange("(a p) o -> p a o", p=128))
            wpT_sb = wp.tile([128, 8, C], F32R, name="wpT_sb")
            nc.sync.dma_start(out=wpT_sb[:], in_=wpT.rearrange("(a p) o -> p a o", p=128))
            cosq_sb = wp.tile([128, R], F32, name="cosq_sb")
            nc.sync.dma_start(out=cosq_sb[:], in_=cosq)
            sinq_sb = wp.tile([128, R], F32, name="sinq_sb")
            nc.sync.dma_start(out=sinq_sb[:], in_=sinq)
            cosk_sb = wp.tile([128, NKV], F32, name="cosk_sb")
            nc.sync.dma_start(out=cosk_sb[:], in_=cosk)
            sink_sb = wp.tile([128, NKV], F32, name="sink_sb")
            nc.sync.dma_start(out=sink_sb[:], in_=sink)
            mcomb_sb = wp.tile([128, 2, 512], F32, name="mcomb_sb")
            nc.sync.dma_start(out=mcomb_sb[:], in_=mcomb)
            jb_sb = wp.tile([128, NKB], F32, name="jb_sb")
            nc.sync.dma_start(out=jb_sb[:], in_=jb)
            eee_sb = wp.tile([128, 2], F32R, name="eee_sb")
            nc.sync.dma_start(out=eee_sb[:], in_=eee)
            e2_sb = wp.tile([2, 128], F32R, name="e2_sb")
            nc.sync.dma_start(out=e2_sb[:], in_=e2)
            ones64_sb = wp.tile([1, 64], F32R, name="ones64_sb")
            nc.sync.dma_start(out=ones64_sb[:], in_=ones64)
            epsb_sb = wp.tile([2, 1], F32, name="epsb_sb")
            nc.sync.dma_start(out=epsb_sb[:], in_=epsb)
            rotm_sb = wp.tile([128, 128], F32R, name="rotm_sb")
            nc.sync.dma_start(out=rotm_sb[:], in_=rotm)

            # ---- persistent activations ----
            q2 = acts.tile([128, 8, R], F32R, name="q2")
            k2 = acts.tile([128, 2, NKV], F32R, name="k2")
            vsb = acts.tile([128, NKB, KVH * 65], F32R, name="vsb")
            yT = acts.tile([128, 8, R], F32R, name="yT")
            outsb = acts.tile([128, 2, C], F32, name="outsb")

            vsb4 = vsb.rearrange("p b (h e) -> p b h e", e=65)
            ONE_F32 = 1065353216  # 1.0f bit pattern; f32r memset is not a valid ISA op
            for _tb in range(NKB):
                for _kvh in range(KVH):
                    nc.gpsimd.memset(
                        vsb4[:, _tb, _kvh, 64:65].bitcast(mybir.dt.uint32), ONE_F32)

            # =========== phase A: projections + qk-norm + rope ===========
            with tc.tile_pool(name="ppA", bufs=2, space="PSUM") as ppA:

                def qk_post(psum_x, n, cos_sb, sin_sb, dst):
                    """psum_x [128, n] raw (q or k)T projection; writes
                    rope(norm(.)) into dst [128, n]."""
                    t_sb = sp.tile([128, n], F32R, tag="tqk", name="t_sb")
                    nc.scalar.copy(t_sb[:], psum_x[:])
                    sq = sp.tile([128, n], F32R, tag="tsq", name="sq")
                    nc.scalar.activation(sq[:], t_sb[:], AF.Square)
                    ss = ppA.tile([2, n], F32, tag="ss", name="ss")
                    for j0 in range(0, n, 512):
                        j1 = min(j0 + 512, n)
                        nc.tensor.matmul(
                            ss[:, j0:j1],
                            eee_sb[:],
                            sq[:, j0:j1],
                            start=True, stop=True)
                    nrm = sp.tile([2, n], F32, tag="nrm", bufs=2, name="nrm")
                    nc.scalar.activation(nrm[:], ss[:], AF.Sqrt, bias=epsb_sb[:])
                    rn = sp.tile([2, n], F32R, tag="trnr", bufs=2, name="rn")
                    with nc.allow_low_precision(reason="f32r feeds the broadcast matmul"):
                        nc.vector.reciprocal(rn[:], nrm[:])
                    # broadcast the two per-head norm rows across partition
                    # halves via a k=2 matmul (gpsimd partition_broadcast is
                    # unreliable on HW)
                    RN = ppA.tile([128, n], F32, tag="ss", name="RN")
                    for j0 in range(0, n, 512):
                        j1 = min(j0 + 512, n)
                        nc.tensor.matmul(
                            RN[:, j0:j1], e2_sb[:], rn[:, j0:j1],
                            start=True, stop=True)
                    rot = ppA.tile([128, n], F32, tag="pj", name="rot")
                    for j0 in range(0, n, 512):
                        j1 = min(j0 + 512, n)
                        nc.tensor.matmul(
                            rot[:, j0:j1],
                            rotm_sb[:],
                            t_sb[:, j0:j1],
                            start=True, stop=True)
                    t1 = sp.tile([128, n], F32, tag="tr1", name="t1")
                    nc.vector.tensor_mul(t1[:], rot[:], sin_sb[:])
                    t2 = sp.tile([128, n], F32, tag="tr2", name="t2")
                    nc.vector.tensor_mul(t2[:], t_sb[:], cos_sb[:])
                    nc.vector.tensor_add(t1[:], t1[:], t2[:])
                    nc.vector.tensor_mul(dst, t1[:], RN[:])

                # Q projection -> q2 (transposed layout, per o-block)
                for ob in range(8):
                    pq = ppA.tile([128, R], F32, tag="pj", name="pq")
                    for ci in range(8):
                        nc.tensor.matmul(
                            pq[:],
                            wqT_sb[:, ci, ob * 128:(ob + 1) * 128],
                            xT_sb[:, ci, WIN:NKV],
                            start=(ci == 0), stop=(ci == 7))
                    qk_post(pq, R, cosq_sb, sinq_sb, q2[:, ob, :])

                # K projection -> k2
                for kob in range(2):
                    pk = ppA.tile([128, NKV], F32, tag="pj", name="pk")
                    for ci in range(8):
                        for j0 in range(0, NKV, 512):
                            j1 = min(j0 + 512, NKV)
                            nc.tensor.matmul(
                                pk[:, j0:j1],
                                wkvT_sb[:, ci, 256 + kob * 128:256 + (kob + 1) * 128],
                                xT_sb[:, ci, j0:j1],
                                start=(ci == 0), stop=(ci == 7))
                    qk_post(pk, NKV, cosk_sb, sink_sb, k2[:, kob, :])

                # V projection -> vsb (natural layout)
                for tb in range(NKB):
                    pvp = ppA.tile([128, 256], F32, tag="pj", name="pvp")
                    for ci in range(8):
                        nc.tensor.matmul(
                            pvp[:],
                            xT_sb[:, ci, tb * 128:(tb + 1) * 128],
                            wkvT_sb[:, ci, 0:256],
                            start=(ci == 0), stop=(ci == 7))
                    nc.vector.tensor_copy(
                        vsb4[:, tb, :, 0:64],
                        pvp.rearrange("p (h e) -> p h e", e=64))

            # =========== phase B: attention + c_proj ===========
            with tc.tile_pool(name="ppB", bufs=1, space="PSUM") as ppB:
                for b in range(8):
                  for s in range(2):
                    h = HPERM[b][s]
                    kvh = h // 4
                    kob, poff = kvh // 2, s * 64
                    qsl = q2[s * 64:s * 64 + 64, b, :]
                    py = ppB.tile([65, R], F32, tag="pv", bufs=3, name="py")
                    for pr in range(3):          # kb pairs (0,1) (2,3) (4,5)
                        ps = ppB.tile([128, 2, R], F32, tag="ps", bufs=3, name="ps")
                        for kl in range(2):
                            kb = pr * 2 + kl
                            nc.tensor.matmul(
                                ps[:, kl, :],
                                k2[poff:poff + 64, kob, kb * 128:(kb + 1) * 128],
                                qsl,
                                start=True, stop=True)
                        psf = ps.rearrange("p a i -> p (a i)")
                        u = sp.tile([128, 2 * R], F32R, tag="uT", bufs=3, name="u")
                        if pr == 1:
                            # inside the window: only the per-key padding bias
                            # (nonzero on cores 0-1 only) applies
                            for kl in range(2):
                                kb = pr * 2 + kl
                                nc.scalar.activation(
                                    u[:, kl * R:(kl + 1) * R], ps[:, kl, :],
                                    AF.Exp, bias=jb_sb[:, kb:kb + 1], scale=0.125)
                        else:
                            stg = sp.tile([128, 2 * R], F32, tag="stg", name="stg")
                            nc.vector.tensor_add(stg[:], psf, mcomb_sb[:, pr // 2, :])
                            nc.scalar.activation(u[:], stg[:], AF.Exp, scale=0.125)
                        if dbg and pr == 0:
                            nc.sync.dma_start(out=dbg["du"][h].bitcast(F32R), in_=u[:])
                        for kl in range(2):
                            kb = pr * 2 + kl
                            nc.tensor.matmul(
                                py[:],
                                vsb4[:, kb, kvh, :],
                                u[:, kl * R:(kl + 1) * R],
                                start=(kb == 0), stop=(kb == NKB - 1))
                    rsr = sp.tile([1, R], F32R, tag="rsr", name="rsr")
                    with nc.allow_low_precision(reason="f32r feeds the broadcast matmul"):
                        nc.vector.reciprocal(rsr[:], py[64:65, :])
                    RRp = ppB.tile([64, R], F32, tag="rr", bufs=1, name="RRp")
                    nc.tensor.matmul(RRp[:], ones64_sb[:], rsr[:],
                                     start=True, stop=True)
                    RRb = sp.tile([64, R], F32, tag="RRb", name="RRb")
                    nc.scalar.copy(RRb[:], RRp[:])
                    nc.vector.tensor_mul(
                        yT[s * 64:s * 64 + 64, b, :],
                        py[0:64, :], RRb[:])

                if dbg:
                    nc.sync.dma_start(out=dbg["dq2"].bitcast(F32R), in_=q2[:])
                    nc.sync.dma_start(out=dbg["dk2"].bitcast(F32R), in_=k2[:])
                    nc.sync.dma_start(out=dbg["dvsb"].bitcast(F32R), in_=vsb[:])
                    nc.sync.dma_start(out=dbg["dyT"].bitcast(F32R), in_=yT[:])
                # c_proj
                for itb in range(2):
                    for oh in range(2):
                        po = ppB.tile([128, 512], F32, tag="co", bufs=1, name="po")
                        for fb in range(8):
                            nc.tensor.matmul(
                                po[:],
                                yT[:, fb, itb * 128:(itb + 1) * 128],
                                wpT_sb[:, fb, oh * 512:(oh + 1) * 512],
                                start=(fb == 0), stop=(fb == 7))
                        nc.scalar.copy(outsb[:, itb, oh * 512:(oh + 1) * 512], po[:])
                    nc.sync.dma_start(
                        out=out[itb * 128:(itb + 1) * 128, :],
                        in_=outsb[:, itb, :])
    return nc


_CACHE = {}


def _build():
    if "nc" not in _CACHE:
        nc = bacc.Bacc(trn_type="TRN2", target_bir_lowering=False, debug=False)
        _emit(nc)
        nc.compile()
        _CACHE["nc"] = nc
    return _CACHE["nc"]


def _make_core_inputs(x, Wq, Wkv, Wproj):
    """Host-side prep: per-core shards + constant tables (all float32)."""
    x2 = np.ascontiguousarray(np.asarray(x, dtype=np.float32).reshape(T, C))
    # head permutation (see HPERM): feature index new = (2b+s)*64+d <- old h*64+d
    perm = np.concatenate([
        np.arange(64) + HPERM[b][s] * 64
        for b in range(8) for s in range(2)])
    wqT = np.ascontiguousarray(np.asarray(Wq, dtype=np.float32).T[:, perm])
    # Wkv rows: [k(4h x 64) | v(4h x 64)]; wkvT columns reordered to [v | k]
    wkv = np.asarray(Wkv, dtype=np.float32)
    wkvT = np.ascontiguousarray(np.concatenate([wkv[256:512], wkv[0:256]], axis=0).T)
    wpT = np.ascontiguousarray(np.asarray(Wproj, dtype=np.float32).T[perm, :])
    cos, sin = _rope_tables()

    # rotate-half matrix (lhsT layout [k, m]): out[m] = sum_k rotm[k, m] * q[k]
    rot = np.zeros((64, 64), dtype=np.float32)
    for m in range(32):
        rot[m + 32, m] = -1.0       # out[m] = -q[m+32]
        rot[m, m + 32] = 1.0        # out[m+32] = q[m]
    rotm = np.zeros((128, 128), dtype=np.float32)
    rotm[0:64, 0:64] = rot
    rotm[64:128, 64:128] = rot

    eee = np.zeros((128, 2), dtype=np.float32)
    eee[0:64, 0] = 1.0
    eee[64:128, 1] = 1.0
    e2 = np.zeros((2, 128), dtype=np.float32)
    e2[0, 0:64] = 1.0
    e2[1, 64:128] = 1.0
    ones64 = np.ones((1, 64), dtype=np.float32)

    jj = np.arange(128)[:, None]
    ii = np.arange(128)[None, :]
    triA = np.where(jj > ii, 0.0, NEG).astype(np.float32)   # valid: j' > i'
    triB = np.where(jj <= ii, 0.0, NEG).astype(np.float32)  # valid: j' <= i'
    full = np.zeros((128, 128), dtype=np.float32)
    negf = np.full((128, 128), NEG, dtype=np.float32)

    in_maps = []
    for c in range(NCORES):
        qs = c * R
        ks = qs - WIN
        pad = max(0, -ks)            # leading zero-padded kv rows
        xpad = np.zeros((NKV, C), dtype=np.float32)
        xpad[pad:, :] = x2[ks + pad:qs + R, :]
        xTc = np.ascontiguousarray(xpad.T)

        cq = np.ascontiguousarray(np.tile(cos[qs:qs + R].T, (2, 1)))   # [128, R]
        sq_ = np.ascontiguousarray(np.tile(sin[qs:qs + R].T, (2, 1)))
        ck = np.zeros((128, NKV), dtype=np.float32)
        sk = np.zeros((128, NKV), dtype=np.float32)
        ck[:, pad:] = np.tile(cos[ks + pad:qs + R].T, (2, 1))
        sk[:, pad:] = np.tile(sin[ks + pad:qs + R].T, (2, 1))

        # combined additive masks for kb pairs (0,1) and (4,5), jbias folded in
        jb8 = np.zeros((128, NKB), dtype=np.float32)
        for kb in range(NKB):
            jb8[:, kb] = np.where(kb * 128 + np.arange(128) < pad, NEG, 0.0)
        m = np.zeros((128, 2, 2, 2, 128), dtype=np.float32)  # [j, pr, kl, qb, i]
        m[:, 0, 0, 0] = triA + jb8[:, 0:1]
        m[:, 0, 0, 1] = negf
        m[:, 0, 1, 0] = full + jb8[:, 1:2]
        m[:, 0, 1, 1] = triA + jb8[:, 1:2]
        m[:, 1, 0, 0] = triB
        m[:, 1, 0, 1] = full
        m[:, 1, 1, 0] = negf
        m[:, 1, 1, 1] = triB
        mcomb = np.ascontiguousarray(m.reshape(128, 2, 512))
        jbu = (jb8 / 8.0).astype(np.float32)   # unscaled bias for the ACT path

        in_maps.append({
            "xT": xTc, "wqT": wqT, "wkvT": wkvT, "wpT": wpT,
            "cosq": cq, "sinq": sq_, "cosk": ck, "sink": sk,
            "mcomb": mcomb, "jb": jbu, "eee": eee, "e2": e2, "ones64": ones64,
            "epsb": np.full((2, 1), EPS * EPS, dtype=np.float32), "rotm": rotm,
        })
    return in_maps


def _run(x, Wq, Wkv, Wproj, trace=False):
    nc = _build()
    in_maps = _make_core_inputs(x, Wq, Wkv, Wproj)
    res = run_bass_kernel_spmd(nc, in_maps, core_ids=list(range(NCORES)), trace=trace)
    outs = [res.results[c]["out"] for c in range(NCORES)]
    full = np.concatenate(outs, axis=0).reshape(1, T, C).astype(np.float32)
    return full, res


def kernel(x, Wq, Wkv, Wproj):
    full, _ = _run(x, Wq, Wkv, Wproj, trace=False)
    return full


# revision 26
# speedup vs baseline: 1.2574x; 1.0209x over previous
"""Sliding-window causal self-attention (GQA + QK L2-norm + RoPE) on 8 TRN2 cores.

Sharding: data-parallel over the sequence dim. Core c computes output rows
[c*256, (c+1)*256). It loads x rows [c*256-512, (c+1)*256) (sliding-window
halo, zero-padded below row 0) and recomputes K/V projections for the halo
locally, so no collectives are needed.

Per-core layouts (all "T-layouts", contraction dim on SBUF partitions):
  xT   [C, 768]     x-halo transposed (host-side)
  q    qT [o=128(2 heads x 64), ob=8, t=256]     (projection emits transposed)
  k    kT [o=128(2 kv heads x 64), kob=2, t=768]
  v    natural [t=128, tb=6, 4*65] (65-strided with a ones column for rowsums)
  scores  sT [j, i] per (head, key-block)  -> softmax denominator comes out of
          the PV matmul via the ones column; normalization is applied to yT.
  y    yT [f=128, fb=8, t=256] feeds c_proj directly.

Softmax skips the max-subtraction: q,k are L2-normalized so |score| <= 0.125
and exp never overflows. Masked positions get -30000 before exp -> exactly 0.
"""

import os

import numpy as np

import concourse.bacc as bacc
import concourse.mybir as mybir
import concourse.tile as tile
from concourse.bass_utils import run_bass_kernel_spmd

F32 = mybir.dt.float32
F32R = mybir.dt.float32r
AF = mybir.ActivationFunctionType
OP = mybir.AluOpType

T, C = 2048, 1024
H, KVH, D = 16, 4, 64
WIN = 512
NCORES = 8
R = T // NCORES          # 256 query rows per core
NKV = R + WIN            # 768 local kv rows
NKB = NKV // 128         # 6 key blocks per strip
EPS = 1e-6
NEG = -240000.0          # additive mask value, pre-scaled by 8 (exp scale=1/8)

# Q-head placement: block b holds heads (HPERM[b][0], HPERM[b][1]) on partition
# halves 0/1. Chosen so a head's partition half equals its kv head's partition
# half ((h//4) % 2), letting the score matmul read q and k at the same base
# partition. Applied host-side as a column permutation of Wq.T / row
# permutation of Wproj.T.
HPERM = [[0, 4], [1, 5], [2, 6], [3, 7], [8, 12], [9, 13], [10, 14], [11, 15]]


def _rope_tables():
    # mirrors reference._rope_tables in float32
    theta = 1.0 / (10000.0 ** (np.arange(0, D, 2, dtype=np.float32) / D))
    pos = np.arange(T, dtype=np.float32)
    freqs = np.outer(pos, theta)                                   # [T, 32]
    cos = np.concatenate([np.cos(freqs), np.cos(freqs)], axis=-1)  # [T, 64]
    sin = np.concatenate([np.sin(freqs), np.sin(freqs)], axis=-1)
    return cos.astype(np.float32), sin.astype(np.float32)


def _emit(nc):
    xT = nc.dram_tensor("xT", [C, NKV], F32R, kind="ExternalInput").ap()
    wqT = nc.dram_tensor("wqT", [C, C], F32R, kind="ExternalInput").ap()
    wkvT = nc.dram_tensor("wkvT", [C, 512], F32R, kind="ExternalInput").ap()
    wpT = nc.dram_tensor("wpT", [C, C], F32R, kind="ExternalInput").ap()
    cosq = nc.dram_tensor("cosq", [128, R], F32, kind="ExternalInput").ap()
    sinq = nc.dram_tensor("sinq", [128, R], F32, kind="ExternalInput").ap()
    cosk = nc.dram_tensor("cosk", [128, NKV], F32, kind="ExternalInput").ap()
    sink = nc.dram_tensor("sink", [128, NKV], F32, kind="ExternalInput").ap()
    mcomb = nc.dram_tensor("mcomb", [128, 2, 512], F32, kind="ExternalInput").ap()
    jb = nc.dram_tensor("jb", [128, NKB], F32, kind="ExternalInput").ap()
    eee = nc.dram_tensor("eee", [128, 2], F32R, kind="ExternalInput").ap()
    e2 = nc.dram_tensor("e2", [2, 128], F32R, kind="ExternalInput").ap()
    ones64 = nc.dram_tensor("ones64", [1, 64], F32R, kind="ExternalInput").ap()
    epsb = nc.dram_tensor("epsb", [2, 1], F32, kind="ExternalInput").ap()
    rotm = nc.dram_tensor("rotm", [128, 128], F32R, kind="ExternalInput").ap()
    out = nc.dram_tensor("out", [R, C], F32, kind="ExternalOutput").ap()
    dbg = {}
    if os.environ.get("KDEBUG"):
        for nm, shp in [("dq2", [128, 8, R]), ("dk2", [128, 2, NKV]),
                        ("dvsb", [128, NKB, KVH * 65]), ("dyT", [128, 8, R]),
                        ("du", [16, 128, 2 * R])]:
            dbg[nm] = nc.dram_tensor(nm, shp, F32, kind="ExternalOutput").ap()

    with tile.TileContext(nc) as tc:
        with (
            tc.tile_pool(name="wp", bufs=1) as wp,
            tc.tile_pool(name="acts", bufs=1) as acts,
            tc.tile_pool(name="sp", bufs=2) as sp,
        ):
            # ---- persistent loads (big tensors chunked so compute can
            # start as soon as the first contraction chunk lands) ----
            xT_sb = wp.tile([128, 8, NKV], F32R, name="xT_sb")
            xT_r = xT.rearrange("(a p) t -> p a t", p=128)
            for ci in range(8):
                nc.sync.dma_start(out=xT_sb[:, ci, :], in_=xT_r[:, ci, :])
            wqT_sb = wp.tile([128, 8, C], F32R, name="wqT_sb")
            wqT_r = wqT.rearrange("(a p) o -> p a o", p=128)
            wkvT_sb = wp.tile([128, 8, 512], F32R, name="wkvT_sb")
            wkvT_r = wkvT.rearrange("(a p) o -> p a o", p=128)
            for ci in range(8):
                nc.sync.dma_start(out=wqT_sb[:, ci, :], in_=wqT_r[:, ci, :])
                nc.sync.dma_start(out=wkvT_sb[:, ci, :], in_=wkvT_r[:, ci, :])
            wpT_sb = wp.tile([128, 8, C], F32R, name="wpT_sb")
            cosq_sb = wp.tile([128, R], F32, name="cosq_sb")
            nc.sync.dma_start(out=cosq_sb[:], in_=cosq)
            sinq_sb = wp.tile([128, R], F32, name="sinq_sb")
            nc.sync.dma_start(out=sinq_sb[:], in_=sinq)
            cosk_sb = wp.tile([128, NKV], F32, name="cosk_sb")
            nc.sync.dma_start(out=cosk_sb[:], in_=cosk)
            sink_sb = wp.tile([128, NKV], F32, name="sink_sb")
            nc.sync.dma_start(out=sink_sb[:], in_=sink)
            mcomb_sb = wp.tile([128, 2, 512], F32, name="mcomb_sb")
            nc.sync.dma_start(out=mcomb_sb[:], in_=mcomb)
            jb_sb = wp.tile([128, NKB], F32, name="jb_sb")
            nc.sync.dma_start(out=jb_sb[:], in_=jb)
            eee_sb = wp.tile([128, 2], F32R, name="eee_sb")
            nc.sync.dma_start(out=eee_sb[:], in_=eee)
            e2_sb = wp.tile([2, 128], F32R, name="e2_sb")
            nc.sync.dma_start(out=e2_sb[:], in_=e2)
            ones64_sb = wp.tile([1, 64], F32R, name="ones64_sb")
            nc.sync.dma_start(out=ones64_sb[:], in_=ones64)
            epsb_sb = wp.tile([2, 1], F32, name="epsb_sb")
            nc.sync.dma_start(out=epsb_sb[:], in_=epsb)
            rotm_sb = wp.tile([128, 128], F32R, name="rotm_sb")
            nc.sync.dma_start(out=rotm_sb[:], in_=rotm)

            # ---- persistent activations ----
            q2 = acts.tile([128, 8, R], F32R, name="q2")
            k2 = acts.tile([128, 2, NKV], F32R, name="k2")
            vsb = acts.tile([128, NKB, KVH * 65], F32R, name="vsb")
            yT = acts.tile([128, 8, R], F32R, name="yT")
            outsb = acts.tile([128, 2, C], F32, name="outsb")

            vsb4 = vsb.rearrange("p b (h e) -> p b h e", e=65)
            ONE_F32 = 1065353216  # 1.0f bit pattern; f32r memset is not a valid ISA op
            for _tb in range(NKB):
                for _kvh in range(KVH):
                    nc.gpsimd.memset(
                        vsb4[:, _tb, _kvh, 64:65].bitcast(mybir.dt.uint32), ONE_F32)

            # =========== phase A: projections + qk-norm + rope ===========
            with tc.tile_pool(name="ppA", bufs=2, space="PSUM") as ppA:

                def qk_post(psum_x, n, cos_sb, sin_sb, dst):
                    """psum_x [128, n] raw (q or k)T projection; writes
                    rope(norm(.)) into dst [128, n]."""
                    t_sb = sp.tile([128, n], F32R, tag="tqk", name="t_sb")
                    nc.scalar.copy(t_sb[:], psum_x[:])
                    sq = sp.tile([128, n], F32R, tag="tsq", name="sq")
                    nc.scalar.activation(sq[:], t_sb[:], AF.Square)
                    ss = ppA.tile([2, n], F32, tag="ss", name="ss")
                    for j0 in range(0, n, 512):
                        j1 = min(j0 + 512, n)
                        nc.tensor.matmul(
                            ss[:, j0:j1],
                            eee_sb[:],
                            sq[:, j0:j1],
                            start=True, stop=True)
                    nrm = sp.tile([2, n], F32, tag="nrm", bufs=2, name="nrm")
                    nc.scalar.activation(nrm[:], ss[:], AF.Sqrt, bias=epsb_sb[:])
                    rn = sp.tile([2, n], F32R, tag="trnr", bufs=2, name="rn")
                    with nc.allow_low_precision(reason="f32r feeds the broadcast matmul"):
                        nc.vector.reciprocal(rn[:], nrm[:])
                    # broadcast the two per-head norm rows across partition
                    # halves via a k=2 matmul (gpsimd partition_broadcast is
                    # unreliable on HW)
                    RN = ppA.tile([128, n], F32, tag="ss", name="RN")
                    for j0 in range(0, n, 512):
                        j1 = min(j0 + 512, n)
                        nc.tensor.matmul(
                            RN[:, j0:j1], e2_sb[:], rn[:, j0:j1],
                            start=True, stop=True)
                    rot = ppA.tile([128, n], F32, tag="pj", name="rot")
                    for j0 in range(0, n, 512):
                        j1 = min(j0 + 512, n)
                        nc.tensor.matmul(
                            rot[:, j0:j1],
                            rotm_sb[:],
                            t_sb[:, j0:j1],
                            start=True, stop=True)
                    t1 = sp.tile([128, n], F32, tag="tr1", name="t1")
                    nc.vector.tensor_mul(t1[:], rot[:], sin_sb[:])
                    t2 = sp.tile([128, n], F32, tag="tr2", name="t2")
                    nc.vector.tensor_mul(t2[:], t_sb[:], cos_sb[:])
                    nc.vector.tensor_add(t1[:], t1[:], t2[:])
                    nc.vector.tensor_mul(dst, t1[:], RN[:])

                # Q projection -> q2 (transposed layout, per o-block)
                for ob in range(8):
                    pq = ppA.tile([128, R], F32, tag="pj", name="pq")
                    for ci in range(8):
                        nc.tensor.matmul(
                            pq[:],
                            wqT_sb[:, ci, ob * 128:(ob + 1) * 128],
                            xT_sb[:, ci, WIN:NKV],
                            start=(ci == 0), stop=(ci == 7))
                    qk_post(pq, R, cosq_sb, sinq_sb, q2[:, ob, :])

                # K projection -> k2
                for kob in range(2):
                    pk = ppA.tile([128, NKV], F32, tag="pj", name="pk")
                    for ci in range(8):
                        for j0 in range(0, NKV, 512):
                            j1 = min(j0 + 512, NKV)
                            nc.tensor.matmul(
                                pk[:, j0:j1],
                                wkvT_sb[:, ci, 256 + kob * 128:256 + (kob + 1) * 128],
                                xT_sb[:, ci, j0:j1],
                                start=(ci == 0), stop=(ci == 7))
                    qk_post(pk, NKV, cosk_sb, sink_sb, k2[:, kob, :])

                # V projection -> vsb (natural layout)
                for tb in range(NKB):
                    pvp = ppA.tile([128, 256], F32, tag="pj", name="pvp")
                    for ci in range(8):
                        nc.tensor.matmul(
                            pvp[:],
                            xT_sb[:, ci, tb * 128:(tb + 1) * 128],
                            wkvT_sb[:, ci, 0:256],
                            start=(ci == 0), stop=(ci == 7))
                    nc.vector.tensor_copy(
                        vsb4[:, tb, :, 0:64],
                        pvp.rearrange("p (h e) -> p h e", e=64))

            # =========== phase B: attention + c_proj ===========
            with tc.tile_pool(name="ppB", bufs=1, space="PSUM") as ppB:
                for b in range(8):
                  for s in range(2):
                    h = HPERM[b][s]
                    kvh = h // 4
                    kob, poff = kvh // 2, s * 64
                    qsl = q2[s * 64:s * 64 + 64, b, :]
                    py = ppB.tile([65, R], F32, tag="pv", bufs=3, name="py")
                    for pr in range(3):          # kb pairs (0,1) (2,3) (4,5)
                        ps = ppB.tile([128, 2, R], F32, tag="ps", bufs=3, name="ps")
                        for kl in range(2):
                            kb = pr * 2 + kl
                            nc.tensor.matmul(
                                ps[:, kl, :],
                                k2[poff:poff + 64, kob, kb * 128:(kb + 1) * 128],
                                qsl,
                                start=True, stop=True)
                        psf = ps.rearrange("p a i -> p (a i)")
                        u = sp.tile([128, 2 * R], F32R, tag="uT", bufs=3, name="u")
                        if pr == 1:
                            # inside the window: only the per-key padding bias
                            # (nonzero on cores 0-1 only) applies
                            for kl in range(2):
                                kb = pr * 2 + kl
                                nc.scalar.activation(
                                    u[:, kl * R:(kl + 1) * R], ps[:, kl, :],
                                    AF.Exp, bias=jb_sb[:, kb:kb + 1], scale=0.125)
                        else:
                            stg = sp.tile([128, 2 * R], F32, tag="stg", name="stg")
                            nc.vector.tensor_add(stg[:], psf, mcomb_sb[:, pr // 2, :])
                            nc.scalar.activation(u[:], stg[:], AF.Exp, scale=0.125)
                        if dbg and pr == 0:
                            nc.sync.dma_start(out=dbg["du"][h].bitcast(F32R), in_=u[:])
                        for kl in range(2):
                            kb = pr * 2 + kl
                            nc.tensor.matmul(
                                py[:],
                                vsb4[:, kb, kvh, :],
                                u[:, kl * R:(kl + 1) * R],
                                start=(kb == 0), stop=(kb == NKB - 1))
                    rsr = sp.tile([1, R], F32R, tag="rsr", name="rsr")
                    with nc.allow_low_precision(reason="f32r feeds the broadcast matmul"):
                        nc.vector.reciprocal(rsr[:], py[64:65, :])
                    RRp = ppB.tile([64, R], F32, tag="rr", bufs=1, name="RRp")
                    nc.tensor.matmul(RRp[:], ones64_sb[:], rsr[:],
                                     start=True, stop=True)
                    RRb = sp.tile([64, R], F32, tag="RRb", name="RRb")
                    nc.scalar.copy(RRb[:], RRp[:])
                    nc.vector.tensor_mul(
                        yT[s * 64:s * 64 + 64, b, :],
                        py[0:64, :], RRb[:])

                if dbg:
                    nc.sync.dma_start(out=dbg["dq2"].bitcast(F32R), in_=q2[:])
                    nc.sync.dma_start(out=dbg["dk2"].bitcast(F32R), in_=k2[:])
                    nc.sync.dma_start(out=dbg["dvsb"].bitcast(F32R), in_=vsb[:])
                    nc.sync.dma_start(out=dbg["dyT"].bitcast(F32R), in_=yT[:])
                # c_proj (wpT loaded late: it is the last tensor needed)
                wpT_r = wpT.rearrange("(a p) o -> p a o", p=128)
                for fb in range(8):
                    nc.sync.dma_start(out=wpT_sb[:, fb, :], in_=wpT_r[:, fb, :])
                for itb in range(2):
                    for oh in range(2):
                        po = ppB.tile([128, 512], F32, tag="co", bufs=1, name="po")
                        for fb in range(8):
                            nc.tensor.matmul(
                                po[:],
                                yT[:, fb, itb * 128:(itb + 1) * 128],
                                wpT_sb[:, fb, oh * 512:(oh + 1) * 512],
                                start=(fb == 0), stop=(fb == 7))
                        nc.scalar.copy(outsb[:, itb, oh * 512:(oh + 1) * 512], po[:])
                    nc.sync.dma_start(
                        out=out[itb * 128:(itb + 1) * 128, :],
                        in_=outsb[:, itb, :])
    return nc


_CACHE = {}


def _build():
    if "nc" not in _CACHE:
        nc = bacc.Bacc(trn_type="TRN2", target_bir_lowering=False, debug=False)
        _emit(nc)
        nc.compile()
        _CACHE["nc"] = nc
    return _CACHE["nc"]


def _make_core_inputs(x, Wq, Wkv, Wproj):
    """Host-side prep: per-core shards + constant tables (all float32)."""
    x2 = np.ascontiguousarray(np.asarray(x, dtype=np.float32).reshape(T, C))
    # head permutation (see HPERM): feature index new = (2b+s)*64+d <- old h*64+d
    perm = np.concatenate([
        np.arange(64) + HPERM[b][s] * 64
        for b in range(8) for s in range(2)])
    wqT = np.ascontiguousarray(np.asarray(Wq, dtype=np.float32).T[:, perm])
    # Wkv rows: [k(4h x 64) | v(4h x 64)]; wkvT columns reordered to [v | k]
    wkv = np.asarray(Wkv, dtype=np.float32)
    wkvT = np.ascontiguousarray(np.concatenate([wkv[256:512], wkv[0:256]], axis=0).T)
    wpT = np.ascontiguousarray(np.asarray(Wproj, dtype=np.float32).T[perm, :])
    cos, sin = _rope_tables()

    # rotate-half matrix (lhsT layout [k, m]): out[m] = sum_k rotm[k, m] * q[k]
    rot = np.zeros((64, 64), dtype=np.float32)
    for m in range(32):
        rot[m + 32, m] = -1.0       # out[m] = -q[m+32]
        rot[m, m + 32] = 1.0        # out[m+32] = q[m]
    rotm = np.zeros((128, 128), dtype=np.float32)
    rotm[0:64, 0:64] = rot
    rotm[64:128, 64:128] = rot

    eee = np.zeros((128, 2), dtype=np.float32)
    eee[0:64, 0] = 1.0
    eee[64:128, 1] = 1.0
    e2 = np.zeros((2, 128), dtype=np.float32)
    e2[0, 0:64] = 1.0
    e2[1, 64:128] = 1.0
    ones64 = np.ones((1, 64), dtype=np.float32)

    jj = np.arange(128)[:, None]
    ii = np.arange(128)[None, :]
    triA = np.where(jj > ii, 0.0, NEG).astype(np.float32)   # valid: j' > i'
    triB = np.where(jj <= ii, 0.0, NEG).astype(np.float32)  # valid: j' <= i'
    full = np.zeros((128, 128), dtype=np.float32)
    negf = np.full((128, 128), NEG, dtype=np.float32)

    in_maps = []
    for c in range(NCORES):
        qs = c * R
        ks = qs - WIN
        pad = max(0, -ks)            # leading zero-padded kv rows
        xpad = np.zeros((NKV, C), dtype=np.float32)
        xpad[pad:, :] = x2[ks + pad:qs + R, :]
        xTc = np.ascontiguousarray(xpad.T)

        cq = np.ascontiguousarray(np.tile(cos[qs:qs + R].T, (2, 1)))   # [128, R]
        sq_ = np.ascontiguousarray(np.tile(sin[qs:qs + R].T, (2, 1)))
        ck = np.zeros((128, NKV), dtype=np.float32)
        sk = np.zeros((128, NKV), dtype=np.float32)
        ck[:, pad:] = np.tile(cos[ks + pad:qs + R].T, (2, 1))
        sk[:, pad:] = np.tile(sin[ks + pad:qs + R].T, (2, 1))

        # combined additive masks for kb pairs (0,1) and (4,5), jbias folded in
        jb8 = np.zeros((128, NKB), dtype=np.float32)
        for kb in range(NKB):
            jb8[:, kb] = np.where(kb * 128 + np.arange(128) < pad, NEG, 0.0)
        m = np.zeros((128, 2, 2, 2, 128), dtype=np.float32)  # [j, pr, kl, qb, i]
        m[:, 0, 0, 0] = triA + jb8[:, 0:1]
        m[:, 0, 0, 1] = negf
        m[:, 0, 1, 0] = full + jb8[:, 1:2]
        m[:, 0, 1, 1] = triA + jb8[:, 1:2]
        m[:, 1, 0, 0] = triB
        m[:, 1, 0, 1] = full
        m[:, 1, 1, 0] = negf
        m[:, 1, 1, 1] = triB
        mcomb = np.ascontiguousarray(m.reshape(128, 2, 512))
        jbu = (jb8 / 8.0).astype(np.float32)   # unscaled bias for the ACT path

        in_maps.append({
            "xT": xTc, "wqT": wqT, "wkvT": wkvT, "wpT": wpT,
            "cosq": cq, "sinq": sq_, "cosk": ck, "sink": sk,
            "mcomb": mcomb, "jb": jbu, "eee": eee, "e2": e2, "ones64": ones64,
            "epsb": np.full((2, 1), EPS * EPS, dtype=np.float32), "rotm": rotm,
        })
    return in_maps


def _run(x, Wq, Wkv, Wproj, trace=False):
    nc = _build()
    in_maps = _make_core_inputs(x, Wq, Wkv, Wproj)
    res = run_bass_kernel_spmd(nc, in_maps, core_ids=list(range(NCORES)), trace=trace)
    outs = [res.results[c]["out"] for c in range(NCORES)]
    full = np.concatenate(outs, axis=0).reshape(1, T, C).astype(np.float32)
    return full, res


def kernel(x, Wq, Wkv, Wproj):
    full, _ = _run(x, Wq, Wkv, Wproj, trace=False)
    return full
